# revision 5
# baseline (speedup 1.0000x reference)
"""ACmix (local 3x3 window attention + dynamic conv mix) on 8 TRN2 NeuronCores.

Sharding: data-parallel over batch B=8, one batch element per core.
Per-core layout: channels (128) on partitions, L = H*W = 4096 on the free dim.

v3: spread-coefficient layout + SBUF->SBUF expansion.
  All 36 per-(head,shift) coefficient rows live at partition PI(h,s)=11s+8h
  of full 128-partition tiles, so the coefficient broadcast (36 rows -> 128
  partitions x 9 shifts) is a single-hop SBUF->SBUF replicate DMA per head
  (3-dim AP, affine in s) instead of a DRAM round trip.  Tails (softmax
  normalize + kern add) and expansions run per 512-column sub-chunk so the
  expansion latency hides under the next quarter's front.
  qkv PSUM->SBUF copies split between scalar (k,v) and vector (q) engines.
"""

import os
from contextlib import ExitStack

import numpy as np
import ml_dtypes

import concourse.bass as bass
import concourse.bacc as bacc
import concourse.tile as tile
from concourse import mybir
from concourse.bass_utils import run_bass_kernel_spmd

BF16 = mybir.dt.bfloat16
F32 = mybir.dt.float32
NPBF16 = ml_dtypes.bfloat16

B, C, H, W = 8, 128, 64, 64
L = H * W                      # 4096
NH, HD, K2 = 4, 32, 9
PW, PH = W + 2, H + 2          # 66
PL = PW * PH                   # 4356
SHIFTS = [(di, dj) for di in (-1, 0, 1) for dj in (-1, 0, 1)]  # s = 3(di+1)+(dj+1)
NQ = 4                         # quarters (1024 cols = 16 image rows each)

# spread-row map: coefficient row for (head h, shift s) lives at partition
# 11*s + 8*h  (36 distinct rows in [0,112]; reads spread across SBUF ports)
def PI(h, s):
    return 11 * s + 8 * h

# weight blob column layout (bf16, 128 partitions)
OFF_QKV = 0             # wqkvT [128, 384]
OFF_PROJ = 384          # wprojT [128, 128]
OFF_WKB = 512           # wkbT' 3 x [128, 128]   (spread out-rows)
OFF_S2 = 896            # S2'  [128, 128]        (per-head shift-sum, spread)
OFF_BONES = 1024        # bones' 9 x [128, 128]  (channel->spread-row routing)
WBLOB = 1024 + 9 * 128  # 2176

GP_ST2 = (0, 1)         # stage-2 product shifts routed to GpSimd
MM_ORDER = [0] + list(range(2, 9)) + [1]  # proj consume order (gp first/last)

TRACE = False
LAST_RESULTS = None


def _ensure_profile_hook():
    """Provide antenv.axon_hooks (missing in this container's antenv stub)
    so run_bass_kernel_spmd(trace=True) can capture NTFF profiles."""
    import sys, types
    try:
        from antenv.axon_hooks import get_axon_ntff_profile_hook  # noqa: F401
        return
    except ImportError:
        pass
    try:
        from trn_agent_boot.trn_boot import _ntff_profile_via_ctypes
        hook = _ntff_profile_via_ctypes("/opt/axon/libaxon_pjrt.so")
    except Exception:
        hook = None
    mod = types.ModuleType("antenv.axon_hooks")
    state = {"hook": hook}
    mod.get_axon_ntff_profile_hook = lambda: state["hook"]
    mod.set_axon_ntff_profile_hook = lambda h: state.__setitem__("hook", h)
    sys.modules["antenv.axon_hooks"] = mod
    import antenv
    antenv.axon_hooks = mod


def _build(subtract_m: bool):
    nc = bacc.Bacc("TRN2", target_bir_lowering=False, debug=False)
    x_ext = nc.declare_dram_parameter("x", [C, L], BF16, isOutput=False)
    wblob_ext = nc.declare_dram_parameter("wblob", [C, WBLOB], BF16, isOutput=False)
    bias_ext = nc.declare_dram_parameter("bias", [C, 1], F32, isOutput=False)
    out_ext = nc.declare_dram_parameter("out", [C, L], BF16, isOutput=True)

    with tile.TileContext(nc) as tc, ExitStack() as ctx:
        pw = ctx.enter_context(tc.tile_pool(name="weights", bufs=1))
        pmain = ctx.enter_context(tc.tile_pool(name="main", bufs=1))
        pce = ctx.enter_context(tc.tile_pool(name="ce", bufs=2))
        psmall = ctx.enter_context(tc.tile_pool(name="small", bufs=2))
        pprod = ctx.enter_context(tc.tile_pool(name="prods", bufs=2))
        pdram = ctx.enter_context(tc.tile_pool(name="dram", bufs=1, space="DRAM"))
        c_dram = pdram.tile([C, L], BF16, name="c_rt")  # spread rows, (qt,sub) cols
        DP = L  # c_dram row pitch (elements)

        # ---- input / weight DMAs (critical pieces first) ------------------
        x_sb = pmain.tile([C, L], BF16)
        wblob = pw.tile([C, WBLOB], BF16)
        bias_sb = pw.tile([C, 1], F32)
        nc.sync.dma_start(wblob[:, 0:384], wblob_ext[:, 0:384])
        nc.sync.dma_start(x_sb[:, 0:1024], x_ext[:, 0:1024])
        nc.sync.dma_start(bias_sb[:], bias_ext[:])
        for xq in range(1, 4):
            nc.sync.dma_start(x_sb[:, xq * 1024 : xq * 1024 + 1024],
                              x_ext[:, xq * 1024 : xq * 1024 + 1024])
        wq = (WBLOB - 384 + 1) // 2
        for wi in range(2):
            a, b = 384 + wi * wq, min(WBLOB, 384 + wi * wq + wq)
            nc.sync.dma_start(wblob[:, a:b], wblob_ext[:, a:b])

        wqkvT = wblob[:, OFF_QKV : OFF_QKV + 384]
        wprojT = wblob[:, OFF_PROJ : OFF_PROJ + 128]
        wkbT = [wblob[:, OFF_WKB + 128 * j : OFF_WKB + 128 * j + 128] for j in range(3)]
        s2m = wblob[:, OFF_S2 : OFF_S2 + 128]
        bones = [wblob[:, OFF_BONES + 128 * s : OFF_BONES + 128 * s + 128]
                 for s in range(9)]

        q_sb = pmain.tile([C, L], BF16)
        k_pad = pmain.tile([C, PL], BF16)
        v_pad = pmain.tile([C, PL], BF16)
        for t in (k_pad, v_pad):
            t3 = t[:].rearrange("p (r c) -> p r c", c=PW)
            nc.gpsimd.memset(t3[:, 0, :], 0.0)          # top padded row
            nc.gpsimd.memset(t3[:, PH - 1, :], 0.0)     # bottom padded row
            nc.gpsimd.memset(t3[:, 1 : PH - 1, 0:1], 0.0)
            nc.gpsimd.memset(t3[:, 1 : PH - 1, PW - 1 : PW], 0.0)

        kp3 = k_pad[:].rearrange("p (r c) -> p r c", c=PW)
        vp3 = v_pad[:].rearrange("p (r c) -> p r c", c=PW)

        out_sb = pmain.tile([C, L], BF16)

        # PSUM pools: psQ(4) + psL(2) + psS(1) + psK(1) = 8 banks; after the
        # qkv phase psQ is scoped out and psC (4 banks) takes its place.
        psL = ctx.enter_context(tc.tile_pool(name="psL", bufs=2, space="PSUM"))
        psS = ctx.enter_context(tc.tile_pool(name="psS", bufs=1, space="PSUM"))
        psK = ctx.enter_context(tc.tile_pool(name="psK", bufs=1, space="PSUM"))
        psQ_ctx = tc.tile_pool(name="psQ", bufs=4, space="PSUM")
        psQ = psQ_ctx.__enter__()

        # ---- Phase A: qkv = w_qkv @ x  (copies: q->vector, k/v->scalar) ---
        def qkv_part(t, hf):
            dst3 = (k_pad if t == 1 else v_pad)[:].rearrange(
                "p (r c) -> p r c", c=PW) if t != 0 else None
            for mc in range(4):
                col = hf * 2048 + mc * 512
                ps = psQ.tile([C, 512], F32, tag="qkv", name=f"qkv{t}_{hf}_{mc}")
                nc.tensor.matmul(ps[:], wqkvT[:, t * C : t * C + C],
                                 x_sb[:, col : col + 512], start=True, stop=True)
                if t == 0:
                    nc.vector.tensor_copy(q_sb[:, col : col + 512], ps[:])
                else:
                    r0 = hf * 32 + mc * 8
                    nc.scalar.copy(
                        dst3[:, 1 + r0 : 1 + r0 + 8, 1 : 1 + W],
                        ps[:].rearrange("p (r c) -> p r c", c=W),
                    )

        # ---- per-quarter pieces ------------------------------------------
        e_qs, kern_qs, c_subs, pr_qs, p2_qs, ce_qs, lg_qs, outps_qs = \
            {}, {}, {}, {}, {}, {}, {}, {}

        def front(qt):
            r0q = qt * 16
            # stage-1 products (DVE)
            prs = {}
            for s in range(9):
                di, dj = SHIFTS[s]
                pr = pprod.tile([C, 1024], BF16, tag=f"pr{s}", name=f"pr{s}_{qt}")
                nc.vector.tensor_mul(
                    pr[:].rearrange("p (r c) -> p r c", c=W),
                    q_sb[:, qt * 1024 : qt * 1024 + 1024].rearrange(
                        "p (r c) -> p r c", c=W),
                    kp3[:, 1 + di + r0q : 1 + di + r0q + 16, 1 + dj : 1 + dj + W],
                )
                prs[s] = pr
            pr_qs[qt] = prs
            # bones: both subs interleaved per shift (one LDWEIGHTS per s)
            e_q = psmall.tile([C, 1024], BF16, tag="e", name=f"e{qt}")
            e_qs[qt] = e_q
            lgs = [psL.tile([C, 512], F32, tag="lg", name=f"lg{qt}_{sub}")
                   for sub in range(2)]
            lg_qs[qt] = lgs
            for s in range(9):
                for sub in range(2):
                    nc.tensor.matmul(
                        lgs[sub][:], bones[s],
                        prs[s][:, sub * 512 : sub * 512 + 512],
                        start=(s == 0), stop=(s == 8), skip_group_check=True,
                    )
            kern_q = psmall.tile([C, 1024], BF16, tag="kern", name=f"kern_q{qt}")
            kern_qs[qt] = kern_q
            for sub in range(2):
                nc.scalar.activation(
                    e_q[:, sub * 512 : sub * 512 + 512], lgs[sub][:],
                    mybir.ActivationFunctionType.Exp,
                )
                # wkb (dynamic conv kernel)
                psk = psK.tile([C, 512], F32, tag="kern", name=f"kern{qt}_{sub}")
                col = qt * 1024 + sub * 512
                rr = r0q + sub * 8
                nc.tensor.matmul(psk[:], wkbT[0], q_sb[:, col : col + 512],
                                 start=True, stop=False, skip_group_check=True)
                nc.tensor.matmul(psk[:], wkbT[1],
                                 kp3[:, 1 + rr : 1 + rr + 8, 1 : 1 + W],
                                 start=False, stop=False, skip_group_check=True)
                nc.tensor.matmul(psk[:], wkbT[2],
                                 vp3[:, 1 + rr : 1 + rr + 8, 1 : 1 + W],
                                 start=False, stop=True, skip_group_check=True)
                nc.scalar.activation(
                    kern_q[:, sub * 512 : sub * 512 + 512], psk[:],
                    mybir.ActivationFunctionType.Identity, bias=bias_sb[:],
                )
                # sums of exp over the 9 shifts of each head (spread rows)
                sm = psS.tile([C, 512], F32, tag="sm", name=f"sm{qt}_{sub}")
                nc.tensor.matmul(
                    sm[:], s2m, e_q[:, sub * 512 : sub * 512 + 512],
                    start=True, stop=True, skip_group_check=True,
                )
                tail(qt, sub, sm)

        def tail(qt, sub, sm):
            e_q, kern_q = e_qs[qt], kern_qs[qt]
            c_s = psmall.tile([C, 512], BF16, tag=f"c{sub}", name=f"c{qt}_{sub}")
            c_subs[(qt, sub)] = c_s
            e_sub = e_q[:, sub * 512 : sub * 512 + 512]
            if subtract_m:
                nc.vector.tensor_copy(c_s[:], kern_q[:, sub * 512 : sub * 512 + 512])
            else:
                r_t = psmall.tile([C, 512], F32, tag="r", name=f"r{qt}_{sub}")
                nc.vector.reciprocal_approx_fast(r_t[:], sm[:])
                nc.vector.tensor_mul(e_sub, e_sub, r_t[:])
                nc.vector.tensor_add(c_s[:], kern_q[:, sub * 512 : sub * 512 + 512],
                                     e_sub)
            # replicate expansion via a DRAM hop: write the 36 live spread
            # rows (PI(h,s)=11s+8h), then per head one affine replicate-read
            # -> 32 partitions x 9 slot-columns of ce.
            if sub == 0:
                ce = pce.tile([C, 9 * 1024], BF16, tag="ce", name=f"ce{qt}")
                ce_qs[qt] = ce
            ce = ce_qs[qt]
            CP = c_s[:].ap[0][0]
            EP = ce[:].ap[0][0]
            cbase = c_dram[:].offset + qt * 1024 + sub * 512
            nc.sync.dma_start(
                bass.AP(c_dram.tensor, cbase, [[DP, 128], [1, 512]]),
                bass.AP(c_s.tensor, c_s[:].offset, [[CP, 128], [1, 512]]),
            )
            for h in range(4):
                eng = nc.sync if h < 2 else nc.scalar
                src = bass.AP(c_dram.tensor, cbase + 8 * h * DP,
                              [[0, 32], [11 * DP, 9], [1, 512]])
                dst = bass.AP(ce.tensor, ce[:].offset + 32 * h * EP + sub * 512,
                              [[EP, 32], [1024, 9], [1, 512]])
                eng.dma_start(dst, src)

        def stage2(qt):
            r0q = qt * 16
            ce = ce_qs[qt]
            p2s = {}
            p2_qs[qt] = p2s

            def p2_mul(s, eng):
                di, dj = SHIFTS[s]
                p2 = pprod.tile([C, 1024], BF16, tag=f"p2_{s}", name=f"p2_{qt}_{s}")
                p2s[s] = p2
                eng.tensor_mul(
                    p2[:].rearrange("p (r c) -> p r c", c=W),
                    ce[:, s * 1024 : s * 1024 + 1024].rearrange(
                        "p (r c) -> p r c", c=W),
                    vp3[:, 1 + di + r0q : 1 + di + r0q + 16, 1 + dj : 1 + dj + W],
                )

            for s in GP_ST2:
                p2_mul(s, nc.gpsimd)
            for s in range(9):
                if s not in GP_ST2:
                    p2_mul(s, nc.vector)
            outps = [psC.tile([C, 512], F32, tag=f"out{sub}", name=f"outps{qt}_{sub}")
                     for sub in range(2)]
            outps_qs[qt] = outps
            for si, s in enumerate(MM_ORDER):
                for sub in range(2):
                    nc.tensor.matmul(
                        outps[sub][:], wprojT,
                        p2s[s][:, sub * 512 : sub * 512 + 512],
                        start=(si == 0), stop=(si == 8),
                        skip_group_check=True,
                    )

        def stage2_out(qt):
            for sub in range(2):
                ci = qt * 2 + sub
                nc.scalar.copy(out_sb[:, ci * 512 : ci * 512 + 512],
                               outps_qs[qt][sub][:])
                nc.scalar.dma_start(
                    out_ext[:, ci * 512 : ci * 512 + 512],
                    out_sb[:, ci * 512 : ci * 512 + 512],
                )

        # ---- pipeline ----------------------------------------------------
        qkv_part(1, 0)          # k h0 (scalar copies)
        qkv_part(0, 0)          # q h0 (vector copies)
        qkv_part(2, 0)          # v h0 (scalar copies)
        front(0)
        qkv_part(1, 1)
        qkv_part(0, 1)
        qkv_part(2, 1)
        psQ_ctx.__exit__(None, None, None)
        psC = ctx.enter_context(tc.tile_pool(name="psC", bufs=2, space="PSUM"))
        front(1)
        stage2(0)
        front(2)
        stage2_out(0)
        stage2(1)
        front(3)
        stage2_out(1)
        stage2(2)
        stage2_out(2)
        stage2(3)
        stage2_out(3)

    nc.compile()
    return nc


_GRAPH_CACHE = {}


def _get_graph(subtract_m: bool):
    if subtract_m not in _GRAPH_CACHE:
        _GRAPH_CACHE[subtract_m] = _build(subtract_m)
    return _GRAPH_CACHE[subtract_m]


def prepare_feeds(x, w_qkv, w_kernel, b_kernel, w_proj, alpha, beta):
    x = np.asarray(x, np.float32)
    w_qkv = np.asarray(w_qkv, np.float32)
    w_kernel = np.asarray(w_kernel, np.float32)
    b_kernel = np.asarray(b_kernel, np.float32)
    w_proj = np.asarray(w_proj, np.float32)
    alpha = float(np.asarray(alpha))
    beta = float(np.asarray(beta))

    # Fold alpha into the output projection and beta/alpha into the kernel
    # branch so the attention coefficient is exactly e/sums on device.
    alpha0 = (alpha == 0.0)
    if alpha0:
        proj_scale, kb_scale = 1.0, beta
    else:
        proj_scale, kb_scale = alpha, beta / alpha

    xb = x.reshape(B, C, L).astype(NPBF16)
    blob = np.zeros((C, WBLOB), np.float32)
    blob[:, OFF_QKV : OFF_QKV + 384] = w_qkv.T
    blob[:, OFF_PROJ : OFF_PROJ + 128] = proj_scale * w_proj.T
    # grouped dynamic-kernel weights, out-rows at spread partitions
    W384 = np.zeros((3 * C, C), np.float32)
    for g in range(NH):
        for k in range(K2):
            W384[96 * g : 96 * g + 96, PI(g, k)] = kb_scale * w_kernel[9 * g + k]
    for j in range(3):
        blob[:, OFF_WKB + 128 * j : OFF_WKB + 128 * j + 128] = \
            W384[128 * j : 128 * j + 128, :]
    for h in range(NH):
        for s in range(K2):
            for sp in range(K2):
                blob[PI(h, sp), OFF_S2 + PI(h, s)] = 1.0
    for s in range(K2):
        for d in range(C):
            blob[d, OFF_BONES + 128 * s + PI(d // 32, s)] = 1.0
    blob = blob.astype(NPBF16)
    bias = np.zeros((C, 1), np.float32)
    for g in range(NH):
        for k in range(K2):
            bias[PI(g, k), 0] = kb_scale * b_kernel[9 * g + k]
    feeds = [
        {"x": np.ascontiguousarray(xb[b]), "wblob": blob, "bias": bias}
        for b in range(B)
    ]
    return feeds, alpha0


def kernel(x, w_qkv, w_kernel, b_kernel, w_proj, alpha, beta):
    global LAST_RESULTS
    in_maps, subtract_m = prepare_feeds(x, w_qkv, w_kernel, b_kernel, w_proj, alpha, beta)
    nc = _get_graph(subtract_m)
    if TRACE:
        _ensure_profile_hook()
    res = run_bass_kernel_spmd(nc, in_maps, list(range(B)), trace=TRACE)
    LAST_RESULTS = res
    out = np.stack([np.asarray(res.results[b]["out"], np.float32) for b in range(B)])
    return out.reshape(B, C, H, W)


# revision 11
# speedup vs baseline: 1.2743x; 1.2743x over previous
"""ACmix (local 3x3 window attention + dynamic conv mix) on 8 TRN2 NeuronCores.

Sharding: data-parallel over batch B=8, one batch element per core.
Per-core layout: channels (128) on partitions, L = H*W = 4096 on the free dim.

v3: spread-coefficient layout + SBUF->SBUF expansion.
  All 36 per-(head,shift) coefficient rows live at partition PI(h,s)=11s+8h
  of full 128-partition tiles, so the coefficient broadcast (36 rows -> 128
  partitions x 9 shifts) is a single-hop SBUF->SBUF replicate DMA per head
  (3-dim AP, affine in s) instead of a DRAM round trip.  Tails (softmax
  normalize + kern add) and expansions run per 512-column sub-chunk so the
  expansion latency hides under the next quarter's front.
  qkv PSUM->SBUF copies split between scalar (k,v) and vector (q) engines.
"""

import os
from contextlib import ExitStack

import numpy as np
import ml_dtypes

import concourse.bass as bass
import concourse.bacc as bacc
import concourse.tile as tile
from concourse import mybir
from concourse.bass_utils import run_bass_kernel_spmd

BF16 = mybir.dt.bfloat16
F32 = mybir.dt.float32
NPBF16 = ml_dtypes.bfloat16

B, C, H, W = 8, 128, 64, 64
L = H * W                      # 4096
NH, HD, K2 = 4, 32, 9
PW, PH = W + 2, H + 2          # 66
PL = PW * PH                   # 4356
SHIFTS = [(di, dj) for di in (-1, 0, 1) for dj in (-1, 0, 1)]  # s = 3(di+1)+(dj+1)
NQ = 4                         # quarters (1024 cols = 16 image rows each)

# spread-row map: coefficient row for (head h, shift s) lives at partition
# 11*s + 8*h  (36 distinct rows in [0,112]; reads spread across SBUF ports)
def PI(h, s):
    return 11 * s + 8 * h

# weight blob column layout (bf16, 128 partitions)
OFF_QKV = 0             # wqkvT [128, 384]
OFF_PROJ = 384          # wprojT [128, 128]
OFF_WKB = 512           # wkbT' 3 x [128, 128]   (spread out-rows)
OFF_S2 = 896            # S2'  [128, 128]        (per-head shift-sum, spread)
OFF_BONES = 1024        # bones' 9 x [128, 128]  (channel->spread-row routing)
WBLOB = 1024 + 9 * 128  # 2176

GP_ST1 = (7, 8)         # stage-1 product shifts routed to GpSimd (consumed last)
GP_ST2 = (0, 1)         # stage-2 product shifts routed to GpSimd (produced first)
BONES_ORDER = list(range(7)) + [7, 8]     # logits accumulation order
MM_ORDER = [0] + list(range(2, 9)) + [1]  # proj consume order (gp first/last)

TRACE = False
LAST_RESULTS = None


def _ensure_profile_hook():
    """Provide antenv.axon_hooks (missing in this container's antenv stub)
    so run_bass_kernel_spmd(trace=True) can capture NTFF profiles."""
    import sys, types
    try:
        from antenv.axon_hooks import get_axon_ntff_profile_hook  # noqa: F401
        return
    except ImportError:
        pass
    try:
        from trn_agent_boot.trn_boot import _ntff_profile_via_ctypes
        hook = _ntff_profile_via_ctypes("/opt/axon/libaxon_pjrt.so")
    except Exception:
        hook = None
    mod = types.ModuleType("antenv.axon_hooks")
    state = {"hook": hook}
    mod.get_axon_ntff_profile_hook = lambda: state["hook"]
    mod.set_axon_ntff_profile_hook = lambda h: state.__setitem__("hook", h)
    sys.modules["antenv.axon_hooks"] = mod
    import antenv
    antenv.axon_hooks = mod


def _build(subtract_m: bool):
    nc = bacc.Bacc("TRN2", target_bir_lowering=False, debug=False)
    x_ext = nc.declare_dram_parameter("x", [C, L], BF16, isOutput=False)
    wblob_ext = nc.declare_dram_parameter("wblob", [C, WBLOB], BF16, isOutput=False)
    bias_ext = nc.declare_dram_parameter("bias", [C, 1], F32, isOutput=False)
    out_ext = nc.declare_dram_parameter("out", [C, L], BF16, isOutput=True)

    with tile.TileContext(nc) as tc, ExitStack() as ctx:
        pw = ctx.enter_context(tc.tile_pool(name="weights", bufs=1))
        pmain = ctx.enter_context(tc.tile_pool(name="main", bufs=1))
        pce = ctx.enter_context(tc.tile_pool(name="ce", bufs=2))
        psmall = ctx.enter_context(tc.tile_pool(name="small", bufs=2))
        pprod = ctx.enter_context(tc.tile_pool(name="prods", bufs=2))
        pdram = ctx.enter_context(tc.tile_pool(name="dram", bufs=1, space="DRAM"))
        c_dram = pdram.tile([C, L], BF16, name="c_rt")  # spread rows, (qt,sub) cols
        DP = L  # c_dram row pitch (elements)

        # ---- input / weight DMAs (critical pieces first) ------------------
        x_sb = pmain.tile([C, L], BF16)
        wblob = pw.tile([C, WBLOB], BF16)
        bias_sb = pw.tile([C, 1], F32)
        nc.sync.dma_start(wblob[:, 0:384], wblob_ext[:, 0:384])
        nc.sync.dma_start(x_sb[:, 0:1024], x_ext[:, 0:1024])
        nc.sync.dma_start(bias_sb[:], bias_ext[:])
        for xq in range(1, 4):
            nc.sync.dma_start(x_sb[:, xq * 1024 : xq * 1024 + 1024],
                              x_ext[:, xq * 1024 : xq * 1024 + 1024])
        wq = (WBLOB - 384 + 1) // 2
        for wi in range(2):
            a, b = 384 + wi * wq, min(WBLOB, 384 + wi * wq + wq)
            nc.sync.dma_start(wblob[:, a:b], wblob_ext[:, a:b])

        wqkvT = wblob[:, OFF_QKV : OFF_QKV + 384]
        wprojT = wblob[:, OFF_PROJ : OFF_PROJ + 128]
        wkbT = [wblob[:, OFF_WKB + 128 * j : OFF_WKB + 128 * j + 128] for j in range(3)]
        s2m = wblob[:, OFF_S2 : OFF_S2 + 128]
        bones = [wblob[:, OFF_BONES + 128 * s : OFF_BONES + 128 * s + 128]
                 for s in range(9)]

        q_sb = pmain.tile([C, L], BF16)
        k_pad = pmain.tile([C, PL], BF16)
        v_pad = pmain.tile([C, PL], BF16)
        for t in (k_pad, v_pad):
            t3 = t[:].rearrange("p (r c) -> p r c", c=PW)
            nc.gpsimd.memset(t3[:, 0, :], 0.0)          # top padded row
            nc.gpsimd.memset(t3[:, PH - 1, :], 0.0)     # bottom padded row
            nc.gpsimd.memset(t3[:, 1 : PH - 1, 0:1], 0.0)
            nc.gpsimd.memset(t3[:, 1 : PH - 1, PW - 1 : PW], 0.0)

        kp3 = k_pad[:].rearrange("p (r c) -> p r c", c=PW)
        vp3 = v_pad[:].rearrange("p (r c) -> p r c", c=PW)

        out_sb = pmain.tile([C, L], BF16)

        # PSUM pools: psQ(4) + psL(2) + psS(1) + psK(1) = 8 banks; after the
        # qkv phase psQ is scoped out and psC (4 banks) takes its place.
        psL = ctx.enter_context(tc.tile_pool(name="psL", bufs=2, space="PSUM"))
        psS = ctx.enter_context(tc.tile_pool(name="psS", bufs=1, space="PSUM"))
        psK = ctx.enter_context(tc.tile_pool(name="psK", bufs=1, space="PSUM"))
        psQ_ctx = tc.tile_pool(name="psQ", bufs=4, space="PSUM")
        psQ = psQ_ctx.__enter__()

        # ---- Phase A: qkv = w_qkv @ x  (PSUM->SBUF copies on scalar) ------
        def qkv_part(t, hf):
            dst3 = (k_pad if t == 1 else v_pad)[:].rearrange(
                "p (r c) -> p r c", c=PW) if t != 0 else None
            for mc in range(4):
                col = hf * 2048 + mc * 512
                ps = psQ.tile([C, 512], F32, tag="qkv", name=f"qkv{t}_{hf}_{mc}")
                nc.tensor.matmul(ps[:], wqkvT[:, t * C : t * C + C],
                                 x_sb[:, col : col + 512], start=True, stop=True)
                if t == 0:
                    nc.scalar.copy(q_sb[:, col : col + 512], ps[:])
                else:
                    r0 = hf * 32 + mc * 8
                    nc.scalar.copy(
                        dst3[:, 1 + r0 : 1 + r0 + 8, 1 : 1 + W],
                        ps[:].rearrange("p (r c) -> p r c", c=W),
                    )

        # ---- per-quarter pieces ------------------------------------------
        e_qs, kern_qs, c_subs, pr_qs, p2_qs, ce_qs, lg_qs, outps_qs = \
            {}, {}, {}, {}, {}, {}, {}, {}

        def st1_gp(qt):
            # late-consumed stage-1 products prefetched on GpSimd
            r0q = qt * 16
            prs = pr_qs.setdefault(qt, {})
            for s in GP_ST1:
                di, dj = SHIFTS[s]
                pr = pprod.tile([C, 1024], BF16, tag=f"pr{s}", name=f"pr{s}_{qt}")
                nc.gpsimd.tensor_mul(
                    pr[:].rearrange("p (r c) -> p r c", c=W),
                    q_sb[:, qt * 1024 : qt * 1024 + 1024].rearrange(
                        "p (r c) -> p r c", c=W),
                    kp3[:, 1 + di + r0q : 1 + di + r0q + 16, 1 + dj : 1 + dj + W],
                )
                prs[s] = pr

        def front(qt):
            r0q = qt * 16
            # stage-1 products (DVE; GP_ST1 shifts prefetched on gpsimd)
            prs = pr_qs.setdefault(qt, {})
            for s in range(9):
                if s in GP_ST1:
                    continue
                di, dj = SHIFTS[s]
                pr = pprod.tile([C, 1024], BF16, tag=f"pr{s}", name=f"pr{s}_{qt}")
                nc.vector.tensor_mul(
                    pr[:].rearrange("p (r c) -> p r c", c=W),
                    q_sb[:, qt * 1024 : qt * 1024 + 1024].rearrange(
                        "p (r c) -> p r c", c=W),
                    kp3[:, 1 + di + r0q : 1 + di + r0q + 16, 1 + dj : 1 + dj + W],
                )
                prs[s] = pr
            # bones: both subs interleaved per shift (one LDWEIGHTS per s)
            e_q = psmall.tile([C, 1024], BF16, tag="e", name=f"e{qt}")
            e_qs[qt] = e_q
            lgs = [psL.tile([C, 512], F32, tag="lg", name=f"lg{qt}_{sub}")
                   for sub in range(2)]
            lg_qs[qt] = lgs
            for si, s in enumerate(BONES_ORDER):
                for sub in range(2):
                    nc.tensor.matmul(
                        lgs[sub][:], bones[s],
                        prs[s][:, sub * 512 : sub * 512 + 512],
                        start=(si == 0), stop=(si == 8), skip_group_check=True,
                    )
            kern_q = psmall.tile([C, 1024], BF16, tag="kern", name=f"kern_q{qt}")
            kern_qs[qt] = kern_q
            for sub in range(2):
                nc.scalar.activation(
                    e_q[:, sub * 512 : sub * 512 + 512], lgs[sub][:],
                    mybir.ActivationFunctionType.Exp,
                )
                # wkb (dynamic conv kernel)
                psk = psK.tile([C, 512], F32, tag="kern", name=f"kern{qt}_{sub}")
                col = qt * 1024 + sub * 512
                rr = r0q + sub * 8
                nc.tensor.matmul(psk[:], wkbT[0], q_sb[:, col : col + 512],
                                 start=True, stop=False, skip_group_check=True)
                nc.tensor.matmul(psk[:], wkbT[1],
                                 kp3[:, 1 + rr : 1 + rr + 8, 1 : 1 + W],
                                 start=False, stop=False, skip_group_check=True)
                nc.tensor.matmul(psk[:], wkbT[2],
                                 vp3[:, 1 + rr : 1 + rr + 8, 1 : 1 + W],
                                 start=False, stop=True, skip_group_check=True)
                nc.scalar.activation(
                    kern_q[:, sub * 512 : sub * 512 + 512], psk[:],
                    mybir.ActivationFunctionType.Identity, bias=bias_sb[:],
                )
                # sums of exp over the 9 shifts of each head (spread rows)
                sm = psS.tile([C, 512], F32, tag="sm", name=f"sm{qt}_{sub}")
                nc.tensor.matmul(
                    sm[:], s2m, e_q[:, sub * 512 : sub * 512 + 512],
                    start=True, stop=True, skip_group_check=True,
                )
                tail(qt, sub, sm)

        def tail(qt, sub, sm):
            e_q, kern_q = e_qs[qt], kern_qs[qt]
            if sub == 0:
                c_q = psmall.tile([C, 1024], BF16, tag="c", name=f"c{qt}")
                c_subs[qt] = c_q
            c_q = c_subs[qt]
            c_s = c_q[:, sub * 512 : sub * 512 + 512]
            e_sub = e_q[:, sub * 512 : sub * 512 + 512]
            if subtract_m:
                nc.vector.tensor_copy(c_s, kern_q[:, sub * 512 : sub * 512 + 512])
            else:
                r_t = psmall.tile([C, 512], F32, tag="r", name=f"r{qt}_{sub}")
                nc.vector.reciprocal_approx_fast(r_t[:], sm[:])
                nc.vector.tensor_mul(e_sub, e_sub, r_t[:])
                nc.vector.tensor_add(c_s, kern_q[:, sub * 512 : sub * 512 + 512],
                                     e_sub)
            if sub == 0:
                return
            # replicate expansion via a DRAM hop (per quarter): write the
            # spread c rows, then per head one affine replicate-read
            # -> 32 partitions x 9 slot-columns of ce.
            ce = pce.tile([C, 9 * 1024], BF16, tag="ce", name=f"ce{qt}")
            ce_qs[qt] = ce
            CP = c_q[:].ap[0][0]
            EP = ce[:].ap[0][0]
            cbase = c_dram[:].offset + qt * 1024
            nc.sync.dma_start(
                bass.AP(c_dram.tensor, cbase, [[DP, 128], [1, 1024]]),
                bass.AP(c_q.tensor, c_q[:].offset, [[CP, 128], [1, 1024]]),
            )
            for h in range(4):
                src = bass.AP(c_dram.tensor, cbase + 8 * h * DP,
                              [[0, 32], [11 * DP, 9], [1, 1024]])
                dst = bass.AP(ce.tensor, ce[:].offset + 32 * h * EP,
                              [[EP, 32], [1024, 9], [1, 1024]])
                nc.sync.dma_start(dst, src)

        def stage2(qt):
            r0q = qt * 16
            ce = ce_qs[qt]
            p2s = {}
            p2_qs[qt] = p2s

            def p2_mul(s, eng):
                di, dj = SHIFTS[s]
                p2 = pprod.tile([C, 1024], BF16, tag=f"p2_{s}", name=f"p2_{qt}_{s}")
                p2s[s] = p2
                eng.tensor_mul(
                    p2[:].rearrange("p (r c) -> p r c", c=W),
                    ce[:, s * 1024 : s * 1024 + 1024].rearrange(
                        "p (r c) -> p r c", c=W),
                    vp3[:, 1 + di + r0q : 1 + di + r0q + 16, 1 + dj : 1 + dj + W],
                )

            for s in GP_ST2:
                p2_mul(s, nc.gpsimd)
            for s in range(9):
                if s not in GP_ST2:
                    p2_mul(s, nc.vector)
            outps = [psC.tile([C, 512], F32, tag=f"out{sub}", name=f"outps{qt}_{sub}")
                     for sub in range(2)]
            outps_qs[qt] = outps
            for si, s in enumerate(MM_ORDER):
                for sub in range(2):
                    nc.tensor.matmul(
                        outps[sub][:], wprojT,
                        p2s[s][:, sub * 512 : sub * 512 + 512],
                        start=(si == 0), stop=(si == 8),
                        skip_group_check=True,
                    )

        def stage2_out(qt):
            for sub in range(2):
                ci = qt * 2 + sub
                nc.scalar.copy(out_sb[:, ci * 512 : ci * 512 + 512],
                               outps_qs[qt][sub][:])
            nc.sync.dma_start(
                out_ext[:, qt * 1024 : qt * 1024 + 1024],
                out_sb[:, qt * 1024 : qt * 1024 + 1024],
            )

        # ---- pipeline ----------------------------------------------------
        qkv_part(1, 0)          # k h0
        qkv_part(0, 0)          # q h0
        qkv_part(2, 0)          # v h0
        st1_gp(0)
        front(0)
        qkv_part(1, 1)
        qkv_part(0, 1)
        qkv_part(2, 1)
        st1_gp(1)
        psQ_ctx.__exit__(None, None, None)
        psC = ctx.enter_context(tc.tile_pool(name="psC", bufs=2, space="PSUM"))
        front(1)
        stage2(0)
        st1_gp(2)
        front(2)
        stage2_out(0)
        stage2(1)
        st1_gp(3)
        front(3)
        stage2_out(1)
        stage2(2)
        stage2_out(2)
        stage2(3)
        stage2_out(3)

    nc.compile()
    return nc


_GRAPH_CACHE = {}


def _get_graph(subtract_m: bool):
    if subtract_m not in _GRAPH_CACHE:
        _GRAPH_CACHE[subtract_m] = _build(subtract_m)
    return _GRAPH_CACHE[subtract_m]


def prepare_feeds(x, w_qkv, w_kernel, b_kernel, w_proj, alpha, beta):
    x = np.asarray(x, np.float32)
    w_qkv = np.asarray(w_qkv, np.float32)
    w_kernel = np.asarray(w_kernel, np.float32)
    b_kernel = np.asarray(b_kernel, np.float32)
    w_proj = np.asarray(w_proj, np.float32)
    alpha = float(np.asarray(alpha))
    beta = float(np.asarray(beta))

    # Fold alpha into the output projection and beta/alpha into the kernel
    # branch so the attention coefficient is exactly e/sums on device.
    alpha0 = (alpha == 0.0)
    if alpha0:
        proj_scale, kb_scale = 1.0, beta
    else:
        proj_scale, kb_scale = alpha, beta / alpha

    xb = x.reshape(B, C, L).astype(NPBF16)
    blob = np.zeros((C, WBLOB), np.float32)
    blob[:, OFF_QKV : OFF_QKV + 384] = w_qkv.T
    blob[:, OFF_PROJ : OFF_PROJ + 128] = proj_scale * w_proj.T
    # grouped dynamic-kernel weights, out-rows at spread partitions
    W384 = np.zeros((3 * C, C), np.float32)
    for g in range(NH):
        for k in range(K2):
            W384[96 * g : 96 * g + 96, PI(g, k)] = kb_scale * w_kernel[9 * g + k]
    for j in range(3):
        blob[:, OFF_WKB + 128 * j : OFF_WKB + 128 * j + 128] = \
            W384[128 * j : 128 * j + 128, :]
    for h in range(NH):
        for s in range(K2):
            for sp in range(K2):
                blob[PI(h, sp), OFF_S2 + PI(h, s)] = 1.0
    for s in range(K2):
        for d in range(C):
            blob[d, OFF_BONES + 128 * s + PI(d // 32, s)] = 1.0
    blob = blob.astype(NPBF16)
    bias = np.zeros((C, 1), np.float32)
    for g in range(NH):
        for k in range(K2):
            bias[PI(g, k), 0] = kb_scale * b_kernel[9 * g + k]
    feeds = [
        {"x": np.ascontiguousarray(xb[b]), "wblob": blob, "bias": bias}
        for b in range(B)
    ]
    return feeds, alpha0


def kernel(x, w_qkv, w_kernel, b_kernel, w_proj, alpha, beta):
    global LAST_RESULTS
    in_maps, subtract_m = prepare_feeds(x, w_qkv, w_kernel, b_kernel, w_proj, alpha, beta)
    nc = _get_graph(subtract_m)
    if TRACE:
        _ensure_profile_hook()
    res = run_bass_kernel_spmd(nc, in_maps, list(range(B)), trace=TRACE)
    LAST_RESULTS = res
    out = np.stack([np.asarray(res.results[b]["out"], np.float32) for b in range(B)])
    return out.reshape(B, C, H, W)


# revision 20
# speedup vs baseline: 1.2915x; 1.0135x over previous
"""ACmix (local 3x3 window attention + dynamic conv mix) on 8 TRN2 NeuronCores.

Sharding: data-parallel over batch B=8, one batch element per core.
Per-core layout: channels (128) on partitions, L = H*W = 4096 on the free dim.

v3: spread-coefficient layout + SBUF->SBUF expansion.
  All 36 per-(head,shift) coefficient rows live at partition PI(h,s)=11s+8h
  of full 128-partition tiles, so the coefficient broadcast (36 rows -> 128
  partitions x 9 shifts) is a single-hop SBUF->SBUF replicate DMA per head
  (3-dim AP, affine in s) instead of a DRAM round trip.  Tails (softmax
  normalize + kern add) and expansions run per 512-column sub-chunk so the
  expansion latency hides under the next quarter's front.
  qkv PSUM->SBUF copies split between scalar (k,v) and vector (q) engines.
"""

import os
from contextlib import ExitStack

import numpy as np
import ml_dtypes

import concourse.bass as bass
import concourse.bacc as bacc
import concourse.tile as tile
from concourse import mybir
from concourse.bass_utils import run_bass_kernel_spmd

BF16 = mybir.dt.bfloat16
F32 = mybir.dt.float32
NPBF16 = ml_dtypes.bfloat16

B, C, H, W = 8, 128, 64, 64
L = H * W                      # 4096
NH, HD, K2 = 4, 32, 9
PW, PH = W + 2, H + 2          # 66
PL = PW * PH                   # 4356
SHIFTS = [(di, dj) for di in (-1, 0, 1) for dj in (-1, 0, 1)]  # s = 3(di+1)+(dj+1)
NQ = 4                         # quarters (1024 cols = 16 image rows each)

# spread-row map: coefficient row for (head h, shift s) lives at partition
# 11*s + 8*h  (36 distinct rows in [0,112]; reads spread across SBUF ports)
def PI(h, s):
    return 11 * s + 8 * h

# weight blob column layout (bf16, 128 partitions)
OFF_QKV = 0             # wqkvT [128, 384]
OFF_PROJ = 384          # wprojT [128, 128]
OFF_WKB = 512           # wkbT' 3 x [128, 128]   (spread out-rows)
OFF_S2 = 896            # S2'  [128, 128]        (per-head shift-sum, spread)
OFF_BONES = 1024        # bones' 9 x [128, 128]  (channel->spread-row routing)
WBLOB = 1024 + 9 * 128  # 2176

GP_ST1 = (7, 8)         # stage-1 product shifts routed to GpSimd (consumed last)
GP_ST2 = (0, 1)         # stage-2 product shifts routed to GpSimd (produced first)
BONES_ORDER = list(range(7)) + [7, 8]     # logits accumulation order
MM_ORDER = [0] + list(range(2, 9)) + [1]  # proj consume order (gp first/last)

TRACE = False
LAST_RESULTS = None


def _ensure_profile_hook():
    """Provide antenv.axon_hooks (missing in this container's antenv stub)
    so run_bass_kernel_spmd(trace=True) can capture NTFF profiles."""
    import sys, types
    try:
        from antenv.axon_hooks import get_axon_ntff_profile_hook  # noqa: F401
        return
    except ImportError:
        pass
    try:
        from trn_agent_boot.trn_boot import _ntff_profile_via_ctypes
        hook = _ntff_profile_via_ctypes("/opt/axon/libaxon_pjrt.so")
    except Exception:
        hook = None
    mod = types.ModuleType("antenv.axon_hooks")
    state = {"hook": hook}
    mod.get_axon_ntff_profile_hook = lambda: state["hook"]
    mod.set_axon_ntff_profile_hook = lambda h: state.__setitem__("hook", h)
    sys.modules["antenv.axon_hooks"] = mod
    import antenv
    antenv.axon_hooks = mod


def _build(subtract_m: bool):
    nc = bacc.Bacc("TRN2", target_bir_lowering=False, debug=False)
    x_ext = nc.declare_dram_parameter("x", [C, L], BF16, isOutput=False)
    wblob_ext = nc.declare_dram_parameter("wblob", [C, WBLOB], BF16, isOutput=False)
    bias_ext = nc.declare_dram_parameter("bias", [C, 1], F32, isOutput=False)
    out_ext = nc.declare_dram_parameter("out", [C, L], BF16, isOutput=True)

    with tile.TileContext(nc) as tc, ExitStack() as ctx:
        pw = ctx.enter_context(tc.tile_pool(name="weights", bufs=1))
        pmain = ctx.enter_context(tc.tile_pool(name="main", bufs=1))
        pce = ctx.enter_context(tc.tile_pool(name="ce", bufs=2))
        psmall = ctx.enter_context(tc.tile_pool(name="small", bufs=2))
        pprod = ctx.enter_context(tc.tile_pool(name="prods", bufs=2))
        pdram = ctx.enter_context(tc.tile_pool(name="dram", bufs=1, space="DRAM"))
        c_dram = pdram.tile([C, L], BF16, name="c_rt")  # spread rows, (qt,sub) cols
        DP = L  # c_dram row pitch (elements)

        # ---- input / weight DMAs (critical pieces first) ------------------
        x_sb = pmain.tile([C, L], BF16)
        wblob = pw.tile([C, WBLOB], BF16)
        bias_sb = pw.tile([C, 1], F32)
        nc.sync.dma_start(wblob[:, 0:384], wblob_ext[:, 0:384])
        nc.sync.dma_start(x_sb[:, 0:1024], x_ext[:, 0:1024])
        nc.sync.dma_start(bias_sb[:], bias_ext[:])
        for xq in range(1, 4):
            nc.sync.dma_start(x_sb[:, xq * 1024 : xq * 1024 + 1024],
                              x_ext[:, xq * 1024 : xq * 1024 + 1024])
        wq = (WBLOB - 384 + 1) // 2
        for wi in range(2):
            a, b = 384 + wi * wq, min(WBLOB, 384 + wi * wq + wq)
            nc.sync.dma_start(wblob[:, a:b], wblob_ext[:, a:b])

        wqkvT = wblob[:, OFF_QKV : OFF_QKV + 384]
        wprojT = wblob[:, OFF_PROJ : OFF_PROJ + 128]
        wkbT = [wblob[:, OFF_WKB + 128 * j : OFF_WKB + 128 * j + 128] for j in range(3)]
        s2m = wblob[:, OFF_S2 : OFF_S2 + 128]
        bones = [wblob[:, OFF_BONES + 128 * s : OFF_BONES + 128 * s + 128]
                 for s in range(9)]

        q_sb = pmain.tile([C, L], BF16)
        k_pad = pmain.tile([C, PL], BF16)
        v_pad = pmain.tile([C, PL], BF16)
        for t in (k_pad, v_pad):
            t3 = t[:].rearrange("p (r c) -> p r c", c=PW)
            nc.gpsimd.memset(t3[:, 0, :], 0.0)          # top padded row
            nc.gpsimd.memset(t3[:, PH - 1, :], 0.0)     # bottom padded row
            nc.gpsimd.memset(t3[:, 1 : PH - 1, 0:1], 0.0)
            nc.gpsimd.memset(t3[:, 1 : PH - 1, PW - 1 : PW], 0.0)

        kp3 = k_pad[:].rearrange("p (r c) -> p r c", c=PW)
        vp3 = v_pad[:].rearrange("p (r c) -> p r c", c=PW)

        out_sb = pmain.tile([C, L], BF16)

        # PSUM pools: psQ(3) + psL(2) + psS(2) + psK(1) = 8 banks; after the
        # qkv phase psQ is scoped out and psC (4 banks) takes its place.
        psL = ctx.enter_context(tc.tile_pool(name="psL", bufs=2, space="PSUM"))
        psS = ctx.enter_context(tc.tile_pool(name="psS", bufs=2, space="PSUM"))
        psK = ctx.enter_context(tc.tile_pool(name="psK", bufs=1, space="PSUM"))
        psQ_ctx = tc.tile_pool(name="psQ", bufs=3, space="PSUM")
        psQ = psQ_ctx.__enter__()

        # ---- Phase A: qkv = w_qkv @ x  (PSUM->SBUF copies on scalar) ------
        def qkv_part(t, hf, mcs=range(4)):
            dst3 = (k_pad if t == 1 else v_pad)[:].rearrange(
                "p (r c) -> p r c", c=PW) if t != 0 else None
            for mc in mcs:
                col = hf * 2048 + mc * 512
                ps = psQ.tile([C, 512], F32, tag="qkv", name=f"qkv{t}_{hf}_{mc}")
                nc.tensor.matmul(ps[:], wqkvT[:, t * C : t * C + C],
                                 x_sb[:, col : col + 512], start=True, stop=True)
                if t == 0:
                    nc.scalar.copy(q_sb[:, col : col + 512], ps[:])
                else:
                    r0 = hf * 32 + mc * 8
                    nc.scalar.copy(
                        dst3[:, 1 + r0 : 1 + r0 + 8, 1 : 1 + W],
                        ps[:].rearrange("p (r c) -> p r c", c=W),
                    )

        # ---- per-quarter pieces ------------------------------------------
        e_qs, kern_qs, sm_qs, pr_qs, p2_qs, ce_qs, lg_qs, outps_qs = \
            {}, {}, {}, {}, {}, {}, {}, {}

        def st1_gp(qt):
            # late-consumed stage-1 products prefetched on GpSimd
            r0q = qt * 16
            prs = pr_qs.setdefault(qt, {})
            for s in GP_ST1:
                di, dj = SHIFTS[s]
                pr = pprod.tile([C, 1024], BF16, tag=f"pr{s}", name=f"pr{s}_{qt}")
                nc.gpsimd.tensor_mul(
                    pr[:].rearrange("p (r c) -> p r c", c=W),
                    q_sb[:, qt * 1024 : qt * 1024 + 1024].rearrange(
                        "p (r c) -> p r c", c=W),
                    kp3[:, 1 + di + r0q : 1 + di + r0q + 16, 1 + dj : 1 + dj + W],
                )
                prs[s] = pr

        def products(qt):
            # stage-1 products (DVE; GP_ST1 shifts prefetched on gpsimd)
            r0q = qt * 16
            prs = pr_qs.setdefault(qt, {})
            for s in range(9):
                if s in GP_ST1:
                    continue
                di, dj = SHIFTS[s]
                pr = pprod.tile([C, 1024], BF16, tag=f"pr{s}", name=f"pr{s}_{qt}")
                nc.vector.tensor_mul(
                    pr[:].rearrange("p (r c) -> p r c", c=W),
                    q_sb[:, qt * 1024 : qt * 1024 + 1024].rearrange(
                        "p (r c) -> p r c", c=W),
                    kp3[:, 1 + di + r0q : 1 + di + r0q + 16, 1 + dj : 1 + dj + W],
                )
                prs[s] = pr

        def front_pe(qt):
            r0q = qt * 16
            prs = pr_qs[qt]
            # bones: both subs interleaved per shift (one LDWEIGHTS per s)
            e_q = psmall.tile([C, 1024], BF16, tag="e", name=f"e{qt}")
            e_qs[qt] = e_q
            lgs = [psL.tile([C, 512], F32, tag="lg", name=f"lg{qt}_{sub}")
                   for sub in range(2)]
            lg_qs[qt] = lgs
            for si, s in enumerate(BONES_ORDER):
                for sub in range(2):
                    nc.tensor.matmul(
                        lgs[sub][:], bones[s],
                        prs[s][:, sub * 512 : sub * 512 + 512],
                        start=(si == 0), stop=(si == 8), skip_group_check=True,
                    )
            kern_q = psmall.tile([C, 1024], BF16, tag="kern", name=f"kern_q{qt}")
            kern_qs[qt] = kern_q
            for sub in range(2):
                nc.scalar.activation(
                    e_q[:, sub * 512 : sub * 512 + 512], lgs[sub][:],
                    mybir.ActivationFunctionType.Exp,
                )
                # wkb (dynamic conv kernel)
                psk = psK.tile([C, 512], F32, tag="kern", name=f"kern{qt}_{sub}")
                col = qt * 1024 + sub * 512
                rr = r0q + sub * 8
                nc.tensor.matmul(psk[:], wkbT[0], q_sb[:, col : col + 512],
                                 start=True, stop=False, skip_group_check=True)
                nc.tensor.matmul(psk[:], wkbT[1],
                                 kp3[:, 1 + rr : 1 + rr + 8, 1 : 1 + W],
                                 start=False, stop=False, skip_group_check=True)
                nc.tensor.matmul(psk[:], wkbT[2],
                                 vp3[:, 1 + rr : 1 + rr + 8, 1 : 1 + W],
                                 start=False, stop=True, skip_group_check=True)
                nc.scalar.activation(
                    kern_q[:, sub * 512 : sub * 512 + 512], psk[:],
                    mybir.ActivationFunctionType.Identity, bias=bias_sb[:],
                )
                # sums of exp over the 9 shifts of each head (spread rows)
                sm = psS.tile([C, 512], F32, tag="sm", name=f"sm{qt}_{sub}")
                nc.tensor.matmul(
                    sm[:], s2m, e_q[:, sub * 512 : sub * 512 + 512],
                    start=True, stop=True, skip_group_check=True,
                )
                sm_qs[(qt, sub)] = sm

        def tails(qt):
            e_q, kern_q = e_qs[qt], kern_qs[qt]
            c_q = psmall.tile([C, 1024], BF16, tag="c", name=f"c{qt}")
            for sub in range(2):
                c_s = c_q[:, sub * 512 : sub * 512 + 512]
                e_sub = e_q[:, sub * 512 : sub * 512 + 512]
                if subtract_m:
                    nc.vector.tensor_copy(c_s,
                                          kern_q[:, sub * 512 : sub * 512 + 512])
                else:
                    r_t = psmall.tile([C, 512], F32, tag="r", name=f"r{qt}_{sub}")
                    nc.vector.reciprocal_approx_fast(r_t[:], sm_qs[(qt, sub)][:])
                    nc.vector.tensor_mul(e_sub, e_sub, r_t[:])
                    nc.vector.tensor_add(
                        c_s, kern_q[:, sub * 512 : sub * 512 + 512], e_sub)
            # replicate expansion via a DRAM hop (per quarter): write the
            # spread c rows, then per head one affine replicate-read
            # -> 32 partitions x 9 slot-columns of ce.  Reads split across
            # the two HWDGE rings (sync: h0/h1, scalar: h2/h3).
            ce = pce.tile([C, 9 * 1024], BF16, tag="ce", name=f"ce{qt}")
            ce_qs[qt] = ce
            CP = c_q[:].ap[0][0]
            EP = ce[:].ap[0][0]
            cbase = c_dram[:].offset + qt * 1024
            nc.sync.dma_start(
                bass.AP(c_dram.tensor, cbase, [[DP, 128], [1, 1024]]),
                bass.AP(c_q.tensor, c_q[:].offset, [[CP, 128], [1, 1024]]),
            )
            for h in range(4):
                src = bass.AP(c_dram.tensor, cbase + 8 * h * DP,
                              [[0, 32], [11 * DP, 9], [1, 1024]])
                dst = bass.AP(ce.tensor, ce[:].offset + 32 * h * EP,
                              [[EP, 32], [1024, 9], [1, 1024]])
                nc.sync.dma_start(dst, src)

        def stage2(qt):
            r0q = qt * 16
            ce = ce_qs[qt]
            p2s = {}
            p2_qs[qt] = p2s

            def p2_mul(s, eng):
                di, dj = SHIFTS[s]
                p2 = pprod.tile([C, 1024], BF16, tag=f"p2_{s}", name=f"p2_{qt}_{s}")
                p2s[s] = p2
                eng.tensor_mul(
                    p2[:].rearrange("p (r c) -> p r c", c=W),
                    ce[:, s * 1024 : s * 1024 + 1024].rearrange(
                        "p (r c) -> p r c", c=W),
                    vp3[:, 1 + di + r0q : 1 + di + r0q + 16, 1 + dj : 1 + dj + W],
                )

            for s in GP_ST2:
                p2_mul(s, nc.gpsimd)
            for s in range(9):
                if s not in GP_ST2:
                    p2_mul(s, nc.vector)
            outps = [psC.tile([C, 512], F32, tag=f"out{sub}", name=f"outps{qt}_{sub}")
                     for sub in range(2)]
            outps_qs[qt] = outps
            for si, s in enumerate(MM_ORDER):
                for sub in range(2):
                    nc.tensor.matmul(
                        outps[sub][:], wprojT,
                        p2s[s][:, sub * 512 : sub * 512 + 512],
                        start=(si == 0), stop=(si == 8),
                        skip_group_check=True,
                    )

        def stage2_out(qt):
            for sub in range(2):
                ci = qt * 2 + sub
                nc.scalar.copy(out_sb[:, ci * 512 : ci * 512 + 512],
                               outps_qs[qt][sub][:])
            nc.sync.dma_start(
                out_ext[:, qt * 1024 : qt * 1024 + 1024],
                out_sb[:, qt * 1024 : qt * 1024 + 1024],
            )

        # ---- pipeline (2-quarter front->stage2 skew) ---------------------
        qkv_part(1, 0)            # k h0
        qkv_part(0, 0)            # q h0
        qkv_part(1, 1, (0,))      # k h1 row-block 32..40 (products/wkb of q1)
        qkv_part(2, 0)            # v h0
        qkv_part(2, 1, (0,))      # v h1 row-block 32..40
        st1_gp(0)
        products(0)
        st1_gp(1)
        front_pe(0)
        qkv_part(1, 1, (1, 2, 3))
        qkv_part(0, 1)
        qkv_part(2, 1, (1, 2, 3))
        products(1)
        psQ_ctx.__exit__(None, None, None)
        psC = ctx.enter_context(tc.tile_pool(name="psC", bufs=1, space="PSUM"))
        tails(0)
        front_pe(1)
        st1_gp(2)
        products(2)
        tails(1)
        front_pe(2)
        stage2(0)
        st1_gp(3)
        products(3)
        tails(2)
        front_pe(3)
        stage2(1)
        stage2_out(0)
        tails(3)
        stage2(2)
        stage2_out(1)
        stage2(3)
        stage2_out(2)
        stage2_out(3)

    nc.compile()
    return nc


_GRAPH_CACHE = {}


def _get_graph(subtract_m: bool):
    if subtract_m not in _GRAPH_CACHE:
        _GRAPH_CACHE[subtract_m] = _build(subtract_m)
    return _GRAPH_CACHE[subtract_m]


def prepare_feeds(x, w_qkv, w_kernel, b_kernel, w_proj, alpha, beta):
    x = np.asarray(x, np.float32)
    w_qkv = np.asarray(w_qkv, np.float32)
    w_kernel = np.asarray(w_kernel, np.float32)
    b_kernel = np.asarray(b_kernel, np.float32)
    w_proj = np.asarray(w_proj, np.float32)
    alpha = float(np.asarray(alpha))
    beta = float(np.asarray(beta))

    # Fold alpha into the output projection and beta/alpha into the kernel
    # branch so the attention coefficient is exactly e/sums on device.
    alpha0 = (alpha == 0.0)
    if alpha0:
        proj_scale, kb_scale = 1.0, beta
    else:
        proj_scale, kb_scale = alpha, beta / alpha

    xb = x.reshape(B, C, L).astype(NPBF16)
    blob = np.zeros((C, WBLOB), np.float32)
    blob[:, OFF_QKV : OFF_QKV + 384] = w_qkv.T
    blob[:, OFF_PROJ : OFF_PROJ + 128] = proj_scale * w_proj.T
    # grouped dynamic-kernel weights, out-rows at spread partitions
    W384 = np.zeros((3 * C, C), np.float32)
    for g in range(NH):
        for k in range(K2):
            W384[96 * g : 96 * g + 96, PI(g, k)] = kb_scale * w_kernel[9 * g + k]
    for j in range(3):
        blob[:, OFF_WKB + 128 * j : OFF_WKB + 128 * j + 128] = \
            W384[128 * j : 128 * j + 128, :]
    for h in range(NH):
        for s in range(K2):
            for sp in range(K2):
                blob[PI(h, sp), OFF_S2 + PI(h, s)] = 1.0
    for s in range(K2):
        for d in range(C):
            blob[d, OFF_BONES + 128 * s + PI(d // 32, s)] = 1.0
    blob = blob.astype(NPBF16)
    bias = np.zeros((C, 1), np.float32)
    for g in range(NH):
        for k in range(K2):
            bias[PI(g, k), 0] = kb_scale * b_kernel[9 * g + k]
    feeds = [
        {"x": np.ascontiguousarray(xb[b]), "wblob": blob, "bias": bias}
        for b in range(B)
    ]
    return feeds, alpha0


def kernel(x, w_qkv, w_kernel, b_kernel, w_proj, alpha, beta):
    global LAST_RESULTS
    in_maps, subtract_m = prepare_feeds(x, w_qkv, w_kernel, b_kernel, w_proj, alpha, beta)
    nc = _get_graph(subtract_m)
    if TRACE:
        _ensure_profile_hook()
    res = run_bass_kernel_spmd(nc, in_maps, list(range(B)), trace=TRACE)
    LAST_RESULTS = res
    out = np.stack([np.asarray(res.results[b]["out"], np.float32) for b in range(B)])
    return out.reshape(B, C, H, W)
